# revision 1
# baseline (speedup 1.0000x reference)
"""Trainium2 Bass kernel for nn_GapDecoder.

Computes gaps[i,j] = proj[i] + proj[j] + b2 where
proj = relu(x @ W1 + b1) @ w2, x: [8192, 512] f32.

Strategy (8 NeuronCores, block-partitioned, collective-free):
  The [8192, 8192] output is an 8x8 grid of [1024, 1024] blocks. Core m
  handles chunk set Lm = {m, m+1, m+2, m+4} (mod 8) and emits the 8
  blocks given by the uniform local pattern
      {(0,0),(0,1),(0,2),(0,3),(1,3),(1,0),(3,1),(3,2)}
  over Lm. One cell per difference delta = Lm[q]-Lm[p] (mod 8) makes the
  union over cores an exact partition of all 64 blocks. Each core reads
  just its 4 x-chunks (8MB, transposed on host so the PE contracts over
  D directly), computes proj for those 4096 rows, broadcasts the
  column-direction proj across partitions with rank-1 PE matmuls, and
  writes each block as 8 chunks of [128, 1024]: DVE tensor_scalar add of
  the per-partition row proj, then a DMA store. 40MB of HBM traffic per
  core (vs 48MB row-sharded) and no cross-core dependency, so staggered
  core starts don't serialize anything.
"""

import sys

sys.path.insert(0, "/opt/trn_rl_repo")

import numpy as np

N, D, H = 8192, 512, 32
NCORES = 8
CHUNK = 1024  # block edge / proj chunk
NLOC = 4  # chunks per core
LROWS = NLOC * CHUNK  # local rows per core
STRIPE = 512  # rows per PE stripe
NSTRIP = LROWS // STRIPE
KCH = D // 128

# local chunk offsets and the block pattern (see module docstring)
LOCAL_OFFS = (0, 1, 2, 4)
PATTERN = ((0, 0), (0, 1), (0, 2), (0, 3), (1, 3), (1, 0), (3, 1), (3, 2))

_state = {}

# Set by run for test harnesses that want profile info (see test.py).
LAST_RESULTS = None


def _build():
    from concourse import bacc, tile, mybir

    f32 = mybir.dt.float32
    nc = bacc.Bacc(
        "TRN2", target_bir_lowering=False, debug=False, num_devices=NCORES
    )

    # host-packed stripe-major layout: row s*128+p holds stripe s's data
    # for partition p, k-major — so a stripe load is 128 x 8KB contiguous
    xT_d = nc.dram_tensor(
        "xT4", [NSTRIP * 128, KCH * STRIPE], f32, kind="ExternalInput"
    )
    w1_d = nc.dram_tensor("W1", [D, H], f32, kind="ExternalInput")
    b1_d = nc.dram_tensor("b1c", [H, 1], f32, kind="ExternalInput")
    w2_d = nc.dram_tensor("w2c", [H, 1], f32, kind="ExternalInput")
    # w2 replicated across 128 columns: matmul(lhsT=w2b, rhs=seqT) puts
    # proj[f] on every partition in one step (the column broadcast)
    w2b_d = nc.dram_tensor("w2b", [H, 128], f32, kind="ExternalInput")
    b2b_d = nc.dram_tensor("b2b", [128, 1], f32, kind="ExternalInput")
    # 8 blocks of [CHUNK, CHUNK], stacked along rows
    out_d = nc.dram_tensor("out", [8 * CHUNK, CHUNK], f32, kind="ExternalOutput")

    with tile.TileContext(nc) as tc:
        with (
            tc.tile_pool(name="const", bufs=1) as cpool,
            tc.tile_pool(name="xkp", bufs=8) as xkpool,
            tc.tile_pool(name="work", bufs=4) as wpool,
            tc.tile_pool(name="big", bufs=12) as bigpool,
            tc.tile_pool(name="psum", bufs=2, space="PSUM") as pspool,
            tc.tile_pool(name="psbc", bufs=2, space="PSUM") as psbc,
        ):
            # ---- constants ----
            w1_sb = cpool.tile([128, KCH, H], f32)
            nc.sync.dma_start(
                w1_sb[:], w1_d.ap().rearrange("(k p) h -> p k h", p=128)
            )
            b1_sb = cpool.tile([H, 1], f32)
            nc.sync.dma_start(b1_sb[:], b1_d.ap())
            w2_sb = cpool.tile([H, 1], f32)
            nc.sync.dma_start(w2_sb[:], w2_d.ap())
            w2b_sb = cpool.tile([H, 128], f32)
            nc.sync.dma_start(w2b_sb[:], w2b_d.ap())
            b2b_sb = cpool.tile([128, 1], f32)
            nc.sync.dma_start(b2b_sb[:], b2b_d.ap())

            # per-partition proj scalars ([128, CHUNK//128] per local chunk)
            projcol = [
                cpool.tile([128, CHUNK // 128], f32, name=f"projcol{i}")
                for i in range(NLOC)
            ]
            bcol = [
                cpool.tile([128, CHUNK], f32, name=f"bcol{i}") for i in range(NLOC)
            ]

            # ---- per chunk: proj stripes, then its broadcast, then every
            # block that just became ready — so output DMAs start as soon
            # as the first chunk's proj exists and overlap later compute.
            COMPUTE_ORDER = (0, 1, 3, 2)
            ready = {loc: i for i, loc in enumerate(COMPUTE_ORDER)}
            emitted = set()

            def emit_block(k):
                # alternate the adds between DVE and ACT so neither queue
                # backs up behind the other chunk-compute work
                p, q = PATTERN[k]
                for g in range(CHUNK // 128):
                    ot = bigpool.tile([128, CHUNK], f32, tag="ot", name="ot")
                    # all block adds on ACT: DVE stays exclusively on the
                    # chunk-compute path so the PE never waits on it
                    nc.scalar.add(ot[:], bcol[q][:], projcol[p][:, g : g + 1])
                    r0 = k * CHUNK + g * 128
                    nc.sync.dma_start(out_d.ap()[r0 : r0 + 128, :], ot[:])

            # preload every stripe up front: all read DMAs must precede all
            # output DMAs in the SP HW-DGE FIFO, or later chunks' reads
            # stall behind earlier stages' writes
            xks = {}
            for loc in COMPUTE_ORDER:
                for half in range(CHUNK // STRIPE):
                    s = loc * (CHUNK // STRIPE) + half
                    xk = xkpool.tile(
                        [128, KCH * STRIPE], f32, tag="xk", name=f"xk{s}"
                    )
                    nc.sync.dma_start(
                        xk[:], xT_d.ap()[s * 128 : (s + 1) * 128, :]
                    )
                    xks[s] = xk

            for loc in COMPUTE_ORDER:
              for half in range(CHUNK // STRIPE):
                s = loc * (CHUNK // STRIPE) + half
                xk = xks[s]
                seqT_ps = pspool.tile([H, STRIPE], f32, tag="seqT")
                for k in range(KCH):
                    nc.tensor.matmul(
                        seqT_ps[:],
                        w1_sb[:, k, :],
                        xk[:, k * STRIPE : (k + 1) * STRIPE],
                        start=(k == 0),
                        stop=(k == KCH - 1),
                    )
                seqT_sb = wpool.tile([H, STRIPE], f32, tag="seqT_sb")
                # relu(x + b1) as a fused DVE op, keeping ACT free for the
                # block adds (and avoiding activation-table switching)
                nc.vector.tensor_scalar(
                    seqT_sb[:],
                    seqT_ps[:],
                    b1_sb[:],
                    0.0,
                    op0=mybir.AluOpType.add,
                    op1=mybir.AluOpType.max,
                )
                # broadcast proj of this stripe across all 128 partitions in
                # one matmul, folding b2 into the psum->sbuf copy
                bc_ps = psbc.tile([128, STRIPE], f32, tag="bc")
                nc.tensor.matmul(bc_ps[:], w2b_sb[:], seqT_sb[:])
                nc.vector.tensor_scalar_add(
                    bcol[loc][:, half * STRIPE : (half + 1) * STRIPE],
                    bc_ps[:],
                    b2b_sb[:],
                )
                # local 2 never appears as a block row; skip its scalars
                for c in range(STRIPE // 128) if loc != 2 else ():
                    pc_ps = pspool.tile([128, 1], f32, tag="pc")
                    nc.tensor.matmul(
                        pc_ps[:],
                        seqT_sb[:, c * 128 : (c + 1) * 128],
                        w2_sb[:],
                    )
                    col = half * (STRIPE // 128) + c
                    nc.vector.tensor_copy(projcol[loc][:, col : col + 1], pc_ps[:])

              # emit blocks one stage behind compute: everything ready as of
              # the PREVIOUS chunk. This keeps the next chunk's PSUM-freeing
              # DVE ops ahead of the block adds in each engine's queue, so
              # the PE never stalls on block-emission progress.
              stage = ready[loc]
              for k in range(8):
                  p, q = PATTERN[k]
                  if k not in emitted and max(ready[p], ready[q]) <= max(
                      stage - 1, 0 if stage == 0 else -1
                  ):
                      emitted.add(k)
                      emit_block(k)

            # drain whatever is left (the last two stages)
            for k in range(8):
                if k not in emitted:
                    emitted.add(k)
                    emit_block(k)

    nc.compile()
    return nc


def kernel(gathered_sequences, W1, b1, w2, b2):
    global LAST_RESULTS
    from concourse import bass_utils

    if "nc" not in _state:
        _state["nc"] = _build()
    nc = _state["nc"]

    x = np.ascontiguousarray(gathered_sequences, dtype=np.float32)
    xT = np.ascontiguousarray(x.T)  # [D, N]
    W1 = np.ascontiguousarray(W1, dtype=np.float32)
    b1c = np.ascontiguousarray(np.reshape(b1, (H, 1)), dtype=np.float32)
    w2c = np.ascontiguousarray(np.reshape(w2, (H, 1)), dtype=np.float32)
    w2b = np.ascontiguousarray(np.repeat(w2c, 128, axis=1))
    b2b = np.full((128, 1), np.reshape(b2, ()), dtype=np.float32)

    in_maps = []
    for m in range(NCORES):
        locs = [(m + a) % NCORES for a in LOCAL_OFFS]
        xT4 = np.concatenate(
            [xT[:, L * CHUNK : (L + 1) * CHUNK] for L in locs], axis=1
        )
        # stripe-major pack: [D, LROWS] -> [NSTRIP*128, KCH*STRIPE] with
        # row s*128+p = (stripe s, partition p), k-major within the row
        xT4p = np.ascontiguousarray(
            xT4.reshape(KCH, 128, NSTRIP, STRIPE)
            .transpose(2, 1, 0, 3)
            .reshape(NSTRIP * 128, KCH * STRIPE)
        )
        in_maps.append(
            {
                "xT4": xT4p,
                "W1": W1,
                "b1c": b1c,
                "w2c": w2c,
                "w2b": w2b,
                "b2b": b2b,
            }
        )

    res = bass_utils.run_bass_kernel_spmd(nc, in_maps, core_ids=list(range(NCORES)))
    LAST_RESULTS = res

    out = np.empty((N, N), dtype=np.float32)
    for m in range(NCORES):
        locs = [(m + a) % NCORES for a in LOCAL_OFFS]
        blocks = res.results[m]["out"]
        for k, (p, q) in enumerate(PATTERN):
            gr, gc = locs[p], locs[q]
            out[gr * CHUNK : (gr + 1) * CHUNK, gc * CHUNK : (gc + 1) * CHUNK] = (
                blocks[k * CHUNK : (k + 1) * CHUNK, :]
            )
    return out



# revision 2
# speedup vs baseline: 1.7386x; 1.7386x over previous
"""Trainium2 Bass kernel for nn_GapDecoder.

Computes gaps[i,j] = proj[i] + proj[j] + b2 where
proj = relu(x @ W1 + b1) @ w2, x: [8192, 512] f32.

Strategy (8 NeuronCores, block-partitioned, collective-free, quantized):
  The [8192, 8192] output is an 8x8 grid of [1024, 1024] blocks and is
  SYMMETRIC (gaps[i,j] = gaps[j,i]), so only ~half the blocks need to be
  materialized on device; the host mirrors the transpose. With the
  2e-2 relative-error budget the output is written as uint8 with a
  per-block scale (max quantization error ~0.4% of max|gaps|), and x is
  read in bf16. Per-core HBM traffic drops from 40MB (f32 baseline) to
  ~8.1MB: 4MB bf16 x read + ~4.1MB u8 writes.

  Core m holds chunks Lm = (m, m+1, m+2, m+4) (mod 8) and emits, in
  LOCAL indices (identical graph on every core — SPMD):
    (0,0) diagonal block, upper-triangle strips only
    (0,1), (0,2), (1,3) full blocks
    (0,3) triangle strips of block (m, m+4); the union of this with the
          partner core's (m+4, m) triangle transposed covers the block.
  Triangle strip g (rows g*128..) spans cols [g*128, 1024): strip-level
  upper triangle; R ∪ R^T of that pattern tiles the full square.

  Quantization: per block (rows chunk p, cols chunk q)
    f = 126.9 / (Gp + Gq + |b2|),  G = max|proj| over the chunk
    u8 = round(f*(proj_i + proj_j + b2) + 128)   in [1.1, 254.9]
  The f values are DMA'd out (fv) so the host dequantizes with the
  exact device scale: gaps = (u8 - 128)/f.
"""

import sys

sys.path.insert(0, "/opt/trn_rl_repo")

import numpy as np

N, D, H = 8192, 512, 32
NCORES = 8
CHUNK = 1024
STRIPE = 512
KCH = D // 128  # 4
NSTRIP = 8  # stripes per core (4 chunks x 2)
QSCALE = 126.9

LOCAL_OFFS = (0, 1, 2, 4)
COMPUTE_ORDER = (0, 1, 3, 2)  # local chunk compute order
# (row_local, col_local, kind, fslot); fulls get outod rows in fslot-1 order
BLOCKS = (
    (0, 0, "tri", 0),  # diagonal (tslot 0)
    (0, 1, "full", 1),
    (0, 2, "full", 2),
    (1, 3, "full", 3),
    (0, 3, "tri", 4),  # d4 half-triangle (tslot 1)
)
# deps: block k emittable once its row & col locals are computed
TRI_OFF = [g * CHUNK - 64 * g * (g - 1) for g in range(9)]  # col offsets; [8]=4608
TRI_W = TRI_OFF[8]

_state = {}
LAST_RESULTS = None


def _build():
    from concourse import bacc, tile, mybir

    f32 = mybir.dt.float32
    bf16 = mybir.dt.bfloat16
    u8 = mybir.dt.uint8
    A = mybir.AluOpType
    AF = mybir.ActivationFunctionType
    AX = mybir.AxisListType

    nc = bacc.Bacc(
        "TRN2", target_bir_lowering=False, debug=False, num_devices=NCORES
    )

    xT_d = nc.dram_tensor(
        "xT4", [NSTRIP * 128, KCH * STRIPE], bf16, kind="ExternalInput"
    )
    w1_d = nc.dram_tensor("W1b", [D, H], bf16, kind="ExternalInput")
    b1_d = nc.dram_tensor("b1c", [H, 1], f32, kind="ExternalInput")
    w2_d = nc.dram_tensor("w2c", [H, 1], bf16, kind="ExternalInput")
    w2b_d = nc.dram_tensor("w2b", [H, 128], bf16, kind="ExternalInput")
    babs_d = nc.dram_tensor("babs", [128, 1], f32, kind="ExternalInput")
    brep_d = nc.dram_tensor("brep", [128, 1], f32, kind="ExternalInput")
    od_d = nc.dram_tensor("outod", [6 * 128, 4096], u8, kind="ExternalOutput")
    tr_d = nc.dram_tensor("outtr", [2 * 128, TRI_W], u8, kind="ExternalOutput")
    fv_d = nc.dram_tensor("fv", [1, 8], f32, kind="ExternalOutput")

    with tile.TileContext(nc) as tc:
        with (
            tc.tile_pool(name="const", bufs=1) as cpool,
            tc.tile_pool(name="xkp", bufs=NSTRIP) as xkpool,
            tc.tile_pool(name="work", bufs=4) as wpool,
            tc.tile_pool(name="big", bufs=4) as bigpool,
            tc.tile_pool(name="tri", bufs=2) as tripool,
            tc.tile_pool(name="psA", bufs=2, space="PSUM") as psA,
            tc.tile_pool(name="psB", bufs=2, space="PSUM") as psB,
            tc.tile_pool(name="psC", bufs=2, space="PSUM") as psC,
        ):
            # ---- constants ----
            w1_sb = cpool.tile([128, KCH, H], bf16)
            nc.sync.dma_start(
                w1_sb[:], w1_d.ap().rearrange("(k p) h -> p k h", p=128)
            )
            b1_sb = cpool.tile([H, 1], f32)
            nc.sync.dma_start(b1_sb[:], b1_d.ap())
            w2_sb = cpool.tile([H, 1], bf16)
            nc.sync.dma_start(w2_sb[:], w2_d.ap())
            w2b_sb = cpool.tile([H, 128], bf16)
            nc.sync.dma_start(w2b_sb[:], w2b_d.ap())
            babs_sb = cpool.tile([128, 1], f32)
            nc.sync.dma_start(babs_sb[:], babs_d.ap())
            brep_sb = cpool.tile([128, 1], f32)
            nc.sync.dma_start(brep_sb[:], brep_d.ap())

            # persistent per-chunk state
            bcol = [
                cpool.tile([128, CHUNK], bf16, name=f"bcol{i}") for i in range(4)
            ]
            projcol = [
                cpool.tile([128, 8], f32, name=f"projcol{i}") for i in range(2)
            ]
            pm = [cpool.tile([128, 2], f32, name=f"pm{i}") for i in range(4)]
            gch = [cpool.tile([128, 1], f32, name=f"g{i}") for i in range(4)]
            fvec = cpool.tile([1, 8], f32)
            nc.vector.memset(fvec[:], 0.0)

            # ---- all read DMAs before any write DMA (SP FIFO order) ----
            xks = {}
            for loc in COMPUTE_ORDER:
                for half in range(2):
                    s = loc * 2 + half
                    xk = xkpool.tile(
                        [128, KCH * STRIPE], bf16, tag="xk", name=f"xk{s}"
                    )
                    nc.sync.dma_start(xk[:], xT_d.ap()[s * 128 : (s + 1) * 128, :])
                    xks[s] = xk

            strip_ctr = [0]

            def strip_op(out_ap, in_ap, f_t, t_col):
                # split the bulk quantize-add ops between DVE and ACT
                k = strip_ctr[0]
                strip_ctr[0] += 1
                if k % 4 == 3:
                    nc.scalar.activation(
                        out_ap, in_ap, AF.Identity, bias=t_col, scale=f_t[:]
                    )
                else:
                    nc.vector.tensor_scalar(
                        out_ap, in_ap, f_t[:], t_col,
                        op0=A.mult, op1=A.add,
                    )

            def emit_block(k):
                lp, lq, kind, fs = BLOCKS[k]
                # scale chain: f = QSCALE / (Gp + Gq + |b2|)
                gsum = cpool.tile([128, 1], f32, name=f"gsum{k}")
                nc.vector.tensor_scalar(
                    gsum[:], gch[lp][:], gch[lq][:], babs_sb[:],
                    op0=A.add, op1=A.add,
                )
                rr = cpool.tile([128, 1], f32, name=f"rr{k}")
                nc.vector.reciprocal(rr[:], gsum[:])
                f_t = cpool.tile([128, 1], f32, name=f"f{k}")
                nc.vector.tensor_scalar_mul(f_t[:], rr[:], QSCALE)
                nc.vector.tensor_copy(fvec[0:1, fs : fs + 1], f_t[0:1, 0:1])
                c2 = cpool.tile([128, 1], f32, name=f"c2{k}")
                nc.vector.tensor_scalar(
                    c2[:], f_t[:], brep_sb[:], 128.0, op0=A.mult, op1=A.add
                )
                t_k = cpool.tile([128, 8], f32, name=f"t{k}")
                nc.vector.tensor_scalar(
                    t_k[:], projcol[lp][:], f_t[:], c2[:], op0=A.mult, op1=A.add
                )

                if kind == "full":
                    b = fs - 1
                    for j in range(2):
                        ot = bigpool.tile([128, 4096], u8, tag="ot", name="ot")
                        for s in range(4):
                            g = 4 * j + s
                            strip_op(
                                ot[:, s * CHUNK : (s + 1) * CHUNK],
                                bcol[lq][:],
                                f_t,
                                t_k[:, g : g + 1],
                            )
                        r0 = (2 * b + j) * 128
                        nc.sync.dma_start(od_d.ap()[r0 : r0 + 128, :], ot[:])
                else:
                    tslot = 0 if k == 0 else 1
                    ot = tripool.tile([128, TRI_W], u8, tag="tri", name="tri")
                    for g in range(8):
                        w = CHUNK - 128 * g
                        strip_op(
                            ot[:, TRI_OFF[g] : TRI_OFF[g] + w],
                            bcol[lq][:, g * 128 : CHUNK],
                            f_t,
                            t_k[:, g : g + 1],
                        )
                    r0 = tslot * 128
                    nc.sync.dma_start(tr_d.ap()[r0 : r0 + 128, :], ot[:])

            done = set()
            emitted = set()
            for loc in COMPUTE_ORDER:
                for half in range(2):
                    s = loc * 2 + half
                    xk = xks[s]
                    seqT_ps = psA.tile([H, STRIPE], f32, tag="seqT")
                    for kk in range(KCH):
                        nc.tensor.matmul(
                            seqT_ps[:],
                            w1_sb[:, kk, :],
                            xk[:, kk * STRIPE : (kk + 1) * STRIPE],
                            start=(kk == 0),
                            stop=(kk == KCH - 1),
                        )
                    seqT_sb = wpool.tile([H, STRIPE], bf16, tag="seqT_sb")
                    # relu(x + b1) on ACT, freeing DVE for the strip ops
                    nc.scalar.activation(
                        seqT_sb[:], seqT_ps[:], AF.Relu, bias=b1_sb[:], scale=1.0
                    )
                    # proj broadcast across partitions (column direction)
                    bc_ps = psB.tile([128, STRIPE], f32, tag="bc")
                    nc.tensor.matmul(bc_ps[:], w2b_sb[:], seqT_sb[:])
                    nc.vector.tensor_copy(
                        bcol[loc][:, half * STRIPE : (half + 1) * STRIPE], bc_ps[:]
                    )
                    nc.vector.reduce_max(
                        pm[loc][:, half : half + 1],
                        bcol[loc][:, half * STRIPE : (half + 1) * STRIPE],
                        axis=AX.X,
                        apply_absolute_value=True,
                    )
                    # per-partition proj scalars (row direction), row locals only
                    if loc < 2:
                        for c in range(4):
                            pc_ps = psC.tile([128, 1], f32, tag="pc")
                            nc.tensor.matmul(
                                pc_ps[:],
                                seqT_sb[:, c * 128 : (c + 1) * 128],
                                w2_sb[:],
                            )
                            g = half * 4 + c
                            nc.vector.tensor_copy(
                                projcol[loc][:, g : g + 1], pc_ps[:]
                            )
                nc.vector.reduce_max(gch[loc][:], pm[loc][:], axis=AX.X)
                done.add(loc)
                for k, (lp, lq, kind, fs) in enumerate(BLOCKS):
                    if k not in emitted and lp in done and lq in done:
                        emitted.add(k)
                        emit_block(k)

            nc.sync.dma_start(fv_d.ap(), fvec[:])

    nc.compile()
    return nc


def _dequant(arr_u8, f):
    return (arr_u8.astype(np.float32) - 128.0) * (1.0 / f)


def _assemble(results):
    """results: list of 8 dicts with outod [768,4096]u8, outtr [256,4608]u8,
    fv [1,8]f32."""
    out = np.empty((N, N), dtype=np.float32)
    ii = (np.arange(CHUNK)[:, None] // 128) * 128
    filled = np.arange(CHUNK)[None, :] >= ii  # triangle strip coverage mask

    def tri_block(r, tslot, f):
        B = np.zeros((CHUNK, CHUNK), dtype=np.float32)
        tr = r["outtr"][tslot * 128 : (tslot + 1) * 128, :]
        for g in range(8):
            w = CHUNK - 128 * g
            B[g * 128 : (g + 1) * 128, g * 128 :] = _dequant(
                tr[:, TRI_OFF[g] : TRI_OFF[g] + w], f
            )
        return B

    for m in range(NCORES):
        locs = [(m + a) % NCORES for a in LOCAL_OFFS]
        r = results[m]
        fv = r["fv"][0]
        # full blocks
        for lp, lq, kind, fs in BLOCKS:
            if kind != "full":
                continue
            b = fs - 1
            raw = r["outod"][2 * b * 128 : (2 * b + 2) * 128, :]
            blk = _dequant(
                raw.reshape(2, 128, 4, CHUNK).transpose(0, 2, 1, 3).reshape(
                    CHUNK, CHUNK
                ),
                fv[fs],
            )
            P, Q = locs[lp], locs[lq]
            out[P * CHUNK : (P + 1) * CHUNK, Q * CHUNK : (Q + 1) * CHUNK] = blk
            out[Q * CHUNK : (Q + 1) * CHUNK, P * CHUNK : (P + 1) * CHUNK] = blk.T
        # diagonal
        B = tri_block(r, 0, fv[0])
        out[m * CHUNK : (m + 1) * CHUNK, m * CHUNK : (m + 1) * CHUNK] = np.where(
            filled, B, B.T
        )
    # d4 pair triangles: block (m, m+4) = core m triangle ∪ core m+4 tri^T
    for m in range(4):
        rA, rB = results[m], results[m + 4]
        BA = tri_block(rA, 1, rA["fv"][0][4])
        BB = tri_block(rB, 1, rB["fv"][0][4])
        X = np.where(filled, BA, BB.T)
        P, Q = m, m + 4
        out[P * CHUNK : (P + 1) * CHUNK, Q * CHUNK : (Q + 1) * CHUNK] = X
        out[Q * CHUNK : (Q + 1) * CHUNK, P * CHUNK : (P + 1) * CHUNK] = X.T
    return out


def kernel(gathered_sequences, W1, b1, w2, b2):
    global LAST_RESULTS
    from concourse import bass_utils
    import ml_dtypes

    bf = ml_dtypes.bfloat16

    if "nc" not in _state:
        _state["nc"] = _build()
    nc = _state["nc"]

    x = np.asarray(gathered_sequences, dtype=np.float32)
    xT = np.ascontiguousarray(x.T).astype(bf)  # [D, N]
    W1b = np.asarray(W1, dtype=np.float32).astype(bf)
    b1c = np.ascontiguousarray(
        np.reshape(np.asarray(b1, np.float32), (H, 1))
    )
    w2c = np.reshape(np.asarray(w2, np.float32), (H, 1)).astype(bf)
    w2b = np.ascontiguousarray(np.repeat(w2c, 128, axis=1))
    b2s = float(np.reshape(np.asarray(b2, np.float32), ()))
    babs = np.full((128, 1), abs(b2s), dtype=np.float32)
    brep = np.full((128, 1), b2s, dtype=np.float32)

    in_maps = []
    for m in range(NCORES):
        locs = [(m + a) % NCORES for a in LOCAL_OFFS]
        xT4 = np.concatenate(
            [xT[:, L * CHUNK : (L + 1) * CHUNK] for L in locs], axis=1
        )
        xT4p = np.ascontiguousarray(
            xT4.reshape(KCH, 128, NSTRIP, STRIPE)
            .transpose(2, 1, 0, 3)
            .reshape(NSTRIP * 128, KCH * STRIPE)
        )
        in_maps.append(
            {
                "xT4": xT4p,
                "W1b": W1b,
                "b1c": b1c,
                "w2c": w2c,
                "w2b": w2b,
                "babs": babs,
                "brep": brep,
            }
        )

    res = bass_utils.run_bass_kernel_spmd(nc, in_maps, core_ids=list(range(NCORES)))
    LAST_RESULTS = res
    return _assemble(res.results)


# revision 4
# speedup vs baseline: 1.9906x; 1.1449x over previous
"""Trainium2 Bass kernel for nn_GapDecoder.

Computes gaps[i,j] = proj[i] + proj[j] + b2 where
proj = relu(x @ W1 + b1) @ w2, x: [8192, 512] f32.

Strategy (8 NeuronCores, block-partitioned, collective-free, quantized):
  gaps is symmetric, so only ~half of the 8x8 grid of [1024,1024] blocks
  is materialized on device (host mirrors transposes), and with the
  2e-2 error budget the output is uint8 with a per-block scale
  (max quant error ~0.4%). x is read in bf16. Per-core HBM traffic is
  ~8.6MB vs 40MB for the f32 row-sharded baseline.

  Core m holds chunks Lm = (m, m+1, m+2, m+4) (mod 8); identical SPMD
  graph per core, blocks in LOCAL indices:
    (0,0) diag triangle  [DVE]   (0,1) full [Pool]   (0,2) full [DVE]
    (1,3) full [ACT]             (0,3) triangle of (m, m+4) [Pool]
  The (0,3) triangle union with the partner core's transposed triangle
  covers that block. Triangle strip g spans cols [g*128, 1024).

  Quantization per block (rows chunk p, cols chunk q):
    f = 126.9/(Gp + Gq + |b2|), G = max|proj| over chunk (from the
    partition-broadcast bcol, so the [128,1] reduce is globally valid)
    u8 = round(f*(proj_i + proj_j + b2) + 128), always in [1.1, 254.9]
  Engines: DVE emits via tmpq = f*bcol + c2 then 1-ALU-op adds (740ns/
  1024 cols vs 970 for 2-op); ACT uses its fused scale+bias form; Pool
  uses 2-op tensor_scalar (its 1-op lowering is pathologically slow).
  Per-engine private input tiles avoid SBUF contention.
"""

import sys

sys.path.insert(0, "/opt/trn_rl_repo")

import numpy as np

N, D, H = 8192, 512, 32
NCORES = 8
CHUNK = 1024
STRIPE = 512
KCH = D // 128  # 4
QSCALE = 126.9

LOCAL_OFFS = (0, 1, 2, 4)
COMPUTE_ORDER = (0, 1, 3, 2)
# (row_local, col_local, kind, fslot)
BLOCKS = (
    (0, 0, "tri", 0),  # diag     DVE   tri slot 0
    (0, 1, "full", 1),  # d1      Pool  odd row 0
    (0, 2, "full", 2),  # d2      DVE   odd row 1
    (1, 3, "full", 4),  # d3      ACT   odd row 2
    (0, 3, "tri", 3),  # d4 half  Pool  tri slot 1
)
TRI_OFF = [g * CHUNK - 64 * g * (g - 1) for g in range(9)]
TRI_W = TRI_OFF[8]  # 4608

_state = {}
LAST_RESULTS = None


def _build():
    from concourse import bacc, tile, mybir

    f32 = mybir.dt.float32
    bf16 = mybir.dt.bfloat16
    u8 = mybir.dt.uint8
    A = mybir.AluOpType
    AF = mybir.ActivationFunctionType
    AX = mybir.AxisListType

    nc = bacc.Bacc(
        "TRN2", target_bir_lowering=False, debug=False, num_devices=NCORES
    )

    xT_d = nc.dram_tensor("xT4", [128, 4 * KCH * CHUNK], bf16, kind="ExternalInput")
    w1_d = nc.dram_tensor("W1b", [D, H], bf16, kind="ExternalInput")
    b1_d = nc.dram_tensor("b1c", [H, 1], f32, kind="ExternalInput")
    w2b_d = nc.dram_tensor("w2b", [H, 128], bf16, kind="ExternalInput")
    babs_d = nc.dram_tensor("babs", [128, 1], f32, kind="ExternalInput")
    brep_d = nc.dram_tensor("brep", [128, 1], f32, kind="ExternalInput")
    od_d = nc.dram_tensor("odd", [3 * 128, 8 * CHUNK], u8, kind="ExternalOutput")
    tr_d = nc.dram_tensor("outtr", [2 * 128, TRI_W], u8, kind="ExternalOutput")
    fv_d = nc.dram_tensor("fv", [1, 8], f32, kind="ExternalOutput")

    with tile.TileContext(nc) as tc:
        with (
            tc.tile_pool(name="const", bufs=1) as cpool,
            tc.tile_pool(name="work", bufs=4) as wpool,
            tc.tile_pool(name="psA", bufs=2, space="PSUM") as psA,
            tc.tile_pool(name="psB", bufs=2, space="PSUM") as psB,
            tc.tile_pool(name="psC", bufs=2, space="PSUM") as psC,
        ):
            # ---- constants ----
            w1_sb = cpool.tile([128, KCH, H], bf16)
            nc.sync.dma_start(
                w1_sb[:], w1_d.ap().rearrange("(k p) h -> p k h", p=128)
            )
            b1_sb = cpool.tile([H, 1], f32)
            nc.sync.dma_start(b1_sb[:], b1_d.ap())
            w2b_sb = cpool.tile([H, 128], bf16)
            nc.sync.dma_start(w2b_sb[:], w2b_d.ap())
            w2_sb = w2b_sb[:, 0:1]
            babs_sb = cpool.tile([128, 1], f32)
            nc.sync.dma_start(babs_sb[:], babs_d.ap())
            brep_sb = cpool.tile([128, 1], f32)
            nc.sync.dma_start(brep_sb[:], brep_d.ap())

            # per-chunk persistent state
            bcol = [cpool.tile([128, CHUNK], bf16, name=f"bcol{i}") for i in range(4)]
            bcol3b = cpool.tile([128, CHUNK], bf16)  # Pool's private copy
            projcol = [cpool.tile([128, 8], f32, name=f"pjc{i}") for i in range(2)]
            pmall = cpool.tile([128, 4], f32)
            # batched per-block scale state: cols = fslot
            gsum = cpool.tile([128, 5], f32)
            rr = cpool.tile([128, 5], f32)
            fsc = cpool.tile([128, 5], f32)
            c2 = cpool.tile([128, 5], f32)
            tks = [cpool.tile([128, 8], f32, name=f"tk{i}") for i in range(5)]
            tmpq = {0: cpool.tile([128, CHUNK], f32, name="tmpq0"),
                    2: cpool.tile([128, CHUNK], f32, name="tmpq2")}
            fvec = cpool.tile([1, 8], f32)
            nc.vector.memset(fvec[:], 1.0)
            # emission tiles (one-shot)
            ot = {1: cpool.tile([128, 8 * CHUNK], u8, name="ot1"),
                  2: cpool.tile([128, 8 * CHUNK], u8, name="ot2"),
                  3: cpool.tile([128, 8 * CHUNK], u8, name="ot3")}
            tri = {0: cpool.tile([128, TRI_W], u8, name="tri0"),
                   4: cpool.tile([128, TRI_W], u8, name="tri4")}

            # ---- all read DMAs first (SP ring: reads before writes) ----
            xks = []
            for loc in range(4):
                xk = cpool.tile([128, KCH * CHUNK], bf16, name=f"xk{loc}")
                c0 = loc * KCH * CHUNK
                nc.sync.dma_start(xk[:], xT_d.ap()[:, c0 : c0 + KCH * CHUNK])
                xks.append(xk)

            def chain_group(slots, in0_ap, in1_ap):
                # gsum cols for the given fslots, then recip/f/c2 in batch
                lo, hi = min(slots), max(slots) + 1
                nc.vector.scalar_tensor_tensor(
                    gsum[:, lo:hi], in0_ap, babs_sb[:], in1_ap,
                    op0=A.add, op1=A.add,
                )
                nc.vector.reciprocal(rr[:, lo:hi], gsum[:, lo:hi])
                nc.vector.tensor_scalar_mul(fsc[:, lo:hi], rr[:, lo:hi], QSCALE)
                nc.vector.tensor_scalar(
                    c2[:, lo:hi], fsc[:, lo:hi], brep_sb[:], 128.0,
                    op0=A.mult, op1=A.add,
                )
                nc.vector.tensor_copy(fvec[0:1, lo:hi], fsc[0:1, lo:hi])

            def emit(k):
                lp, lq, kind, fs = BLOCKS[k]
                fk = fsc[:, fs : fs + 1]
                if k == 0:  # diag, DVE via tmpq + 1-op strips
                    nc.vector.tensor_scalar_mul(tks[0][:], projcol[0][:], fk)
                    nc.vector.tensor_scalar(
                        tmpq[0][:], bcol[0][:], fk, c2[:, 0:1],
                        op0=A.mult, op1=A.add,
                    )
                    for g in range(8):
                        w = CHUNK - 128 * g
                        nc.vector.tensor_scalar_add(
                            tri[0][:, TRI_OFF[g] : TRI_OFF[g] + w],
                            tmpq[0][:, g * 128 :],
                            tks[0][:, g : g + 1],
                        )
                    nc.sync.dma_start(tr_d.ap()[0:128, :], tri[0][:])
                elif k == 2:  # d2, DVE via tmpq + 1-op strips
                    nc.vector.tensor_scalar_mul(tks[2][:], projcol[0][:], fk)
                    nc.vector.tensor_scalar(
                        tmpq[2][:], bcol[2][:], fk, c2[:, 2:3],
                        op0=A.mult, op1=A.add,
                    )
                    for g in range(8):
                        nc.vector.tensor_scalar_add(
                            ot[2][:, g * CHUNK : (g + 1) * CHUNK],
                            tmpq[2][:],
                            tks[2][:, g : g + 1],
                        )
                    nc.sync.dma_start(od_d.ap()[128:256, :], ot[2][:])
                elif k == 3:  # d3, ACT fused scale+bias from raw bcol
                    nc.vector.tensor_scalar(
                        tks[3][:], projcol[1][:], fk, c2[:, 3:4],
                        op0=A.mult, op1=A.add,
                    )
                    for g in range(8):
                        nc.scalar.activation(
                            ot[3][:, g * CHUNK : (g + 1) * CHUNK],
                            bcol[3][:],
                            AF.Identity,
                            bias=tks[3][:, g : g + 1],
                            scale=fk,
                        )
                    nc.sync.dma_start(od_d.ap()[256:384, :], ot[3][:])
                elif k == 1:  # d1, Pool 2-op from raw bcol
                    nc.vector.tensor_scalar(
                        tks[1][:], projcol[0][:], fk, c2[:, 1:2],
                        op0=A.mult, op1=A.add,
                    )
                    for g in range(8):
                        nc.gpsimd.tensor_scalar(
                            ot[1][:, g * CHUNK : (g + 1) * CHUNK],
                            bcol[1][:],
                            fk,
                            tks[1][:, g : g + 1],
                            op0=A.mult,
                            op1=A.add,
                        )
                    nc.sync.dma_start(od_d.ap()[0:128, :], ot[1][:])
                else:  # k == 4: d4h triangle, Pool 2-op from private copy
                    nc.vector.tensor_scalar(
                        tks[4][:], projcol[0][:], fk, c2[:, 4:5],
                        op0=A.mult, op1=A.add,
                    )
                    for g in range(8):
                        w = CHUNK - 128 * g
                        nc.gpsimd.tensor_scalar(
                            tri[4][:, TRI_OFF[g] : TRI_OFF[g] + w],
                            bcol3b[:, g * 128 :],
                            fk,
                            tks[4][:, g : g + 1],
                            op0=A.mult,
                            op1=A.add,
                        )
                    nc.sync.dma_start(tr_d.ap()[128:256, :], tri[4][:])

            pcs = {}
            for loc in COMPUTE_ORDER:
                xk = xks[loc]
                if loc < 2:
                    pcs[loc] = psC.tile([128, 8], f32, tag="pc", name=f"pc{loc}")
                for half in range(2):
                    seqT_ps = psA.tile([H, STRIPE], f32, tag="seqT")
                    for kk in range(KCH):
                        nc.tensor.matmul(
                            seqT_ps[:],
                            w1_sb[:, kk, :],
                            xk[:, kk * CHUNK + half * STRIPE : kk * CHUNK + (half + 1) * STRIPE],
                            start=(kk == 0),
                            stop=(kk == KCH - 1),
                        )
                    seqT_sb = wpool.tile([H, STRIPE], bf16, tag="seqT_sb")
                    nc.scalar.activation(
                        seqT_sb[:], seqT_ps[:], AF.Relu, bias=b1_sb[:], scale=1.0
                    )
                    bc_ps = psB.tile([128, STRIPE], f32, tag="bc")
                    nc.tensor.matmul(bc_ps[:], w2b_sb[:], seqT_sb[:])
                    # psum -> sbuf cast on ACT
                    nc.scalar.activation(
                        bcol[loc][:, half * STRIPE : (half + 1) * STRIPE],
                        bc_ps[:],
                        AF.Copy,
                    )
                    if loc < 2:
                        for c in range(4):
                            g = half * 4 + c
                            nc.tensor.matmul(
                                pcs[loc][:, g : g + 1],
                                seqT_sb[:, c * 128 : (c + 1) * 128],
                                w2_sb,
                                start=True,
                                stop=True,
                            )
                if loc < 2:
                    nc.vector.tensor_copy(projcol[loc][:], pcs[loc][:])
                # global chunk max from the partition-broadcast bcol
                nc.vector.reduce_max(
                    pmall[:, loc : loc + 1],
                    bcol[loc][:],
                    axis=AX.X,
                    apply_absolute_value=True,
                )
                if loc == 3:
                    nc.vector.tensor_copy(bcol3b[:], bcol[3][:])

                # chain groups + emissions at readiness points
                if loc == 1:
                    # diag (G0+G0), d1 (G0+G1): in0 = pm[0] bcast, in1 = pm[0:2]
                    chain_group(
                        (0, 1),
                        pmall[:, 0:1].broadcast_to([128, 2]),
                        pmall[:, 0:2],
                    )
                    emit(0)
                    emit(1)
                elif loc == 3:
                    # slot3=d4h (G0+G3), slot4=d3 (G1+G3): in0 = pm[0:2]
                    chain_group(
                        (3, 4),
                        pmall[:, 0:2],
                        pmall[:, 3:4].broadcast_to([128, 2]),
                    )
                    emit(3)
                    emit(4)
                elif loc == 2:
                    # d2 (G0+G2)
                    chain_group(
                        (2,),
                        pmall[:, 0:1],
                        pmall[:, 2:3],
                    )
                    emit(2)

            nc.sync.dma_start(fv_d.ap(), fvec[:])

    nc.compile()
    return nc


def _dequant(arr_u8, f):
    return (arr_u8.astype(np.float32) - 128.0) * (1.0 / f)


def _assemble(results):
    """results: 8 dicts with odd [384, 8192]u8, outtr [256, 4608]u8,
    fv [1,8]f32."""
    out = np.empty((N, N), dtype=np.float32)
    ii = (np.arange(CHUNK)[:, None] // 128) * 128
    filled = np.arange(CHUNK)[None, :] >= ii

    def tri_block(r, tslot, f):
        B = np.zeros((CHUNK, CHUNK), dtype=np.float32)
        tr = r["outtr"][tslot * 128 : (tslot + 1) * 128, :]
        for g in range(8):
            w = CHUNK - 128 * g
            B[g * 128 : (g + 1) * 128, g * 128 :] = _dequant(
                tr[:, TRI_OFF[g] : TRI_OFF[g] + w], f
            )
        return B

    for m in range(NCORES):
        locs = [(m + a) % NCORES for a in LOCAL_OFFS]
        r = results[m]
        fv = r["fv"][0]
        for lp, lq, kind, fs in BLOCKS:
            if kind != "full":
                continue
            b = {1: 0, 2: 1, 4: 2}[fs]
            raw = r["odd"][b * 128 : (b + 1) * 128, :]
            blk = _dequant(
                raw.reshape(128, 8, CHUNK).swapaxes(0, 1).reshape(CHUNK, CHUNK),
                fv[fs],
            )
            P, Q = locs[lp], locs[lq]
            out[P * CHUNK : (P + 1) * CHUNK, Q * CHUNK : (Q + 1) * CHUNK] = blk
            out[Q * CHUNK : (Q + 1) * CHUNK, P * CHUNK : (P + 1) * CHUNK] = blk.T
        B = tri_block(r, 0, fv[0])
        out[m * CHUNK : (m + 1) * CHUNK, m * CHUNK : (m + 1) * CHUNK] = np.where(
            filled, B, B.T
        )
    for m in range(4):
        rA, rB = results[m], results[m + 4]
        BA = tri_block(rA, 1, rA["fv"][0][3])
        BB = tri_block(rB, 1, rB["fv"][0][3])
        X = np.where(filled, BA, BB.T)
        P, Q = m, m + 4
        out[P * CHUNK : (P + 1) * CHUNK, Q * CHUNK : (Q + 1) * CHUNK] = X
        out[Q * CHUNK : (Q + 1) * CHUNK, P * CHUNK : (P + 1) * CHUNK] = X.T
    return out


def kernel(gathered_sequences, W1, b1, w2, b2):
    global LAST_RESULTS
    from concourse import bass_utils
    import ml_dtypes

    bf = ml_dtypes.bfloat16

    if "nc" not in _state:
        _state["nc"] = _build()
    nc = _state["nc"]

    x = np.asarray(gathered_sequences, dtype=np.float32)
    xT = np.ascontiguousarray(x.T).astype(bf)  # [D, N]
    W1b = np.asarray(W1, dtype=np.float32).astype(bf)
    b1c = np.ascontiguousarray(np.reshape(np.asarray(b1, np.float32), (H, 1)))
    w2c = np.reshape(np.asarray(w2, np.float32), (H, 1)).astype(bf)
    w2b = np.ascontiguousarray(np.repeat(w2c, 128, axis=1))
    b2s = float(np.reshape(np.asarray(b2, np.float32), ()))
    babs = np.full((128, 1), abs(b2s), dtype=np.float32)
    brep = np.full((128, 1), b2s, dtype=np.float32)

    in_maps = []
    for m in range(NCORES):
        locs = [(m + a) % NCORES for a in LOCAL_OFFS]
        # per local chunk: [128, KCH*CHUNK] with cols (k, j); chunks abut
        xT4 = np.concatenate(
            [
                xT[:, L * CHUNK : (L + 1) * CHUNK]
                .reshape(KCH, 128, CHUNK)
                .transpose(1, 0, 2)
                .reshape(128, KCH * CHUNK)
                for L in locs
            ],
            axis=1,
        )
        in_maps.append(
            {
                "xT4": np.ascontiguousarray(xT4),
                "W1b": W1b,
                "b1c": b1c,
                "w2b": w2b,
                "babs": babs,
                "brep": brep,
            }
        )

    res = bass_utils.run_bass_kernel_spmd(nc, in_maps, core_ids=list(range(NCORES)))
    LAST_RESULTS = res
    return _assemble(res.results)
